# revision 20
# baseline (speedup 1.0000x reference)
"""Trainium2 Bass kernel for AnchorGNNPocket (GNN message passing).

Data-parallel over batch B=8: one complex per NeuronCore. Each core runs the
full 4-layer GCL stack on its sample.

v2: edge rows processed in PAIRS ([128, 512] tiles = one PSUM bank), emitted
stage-by-stage over groups of 3 pairs (6 rows) so each engine sees batches of
independent work (software pipelining); edge-MLP matmuls in bf16 (fp32 matmul
is 2-pass LOW_HIGH on trn2); sigmoid batched over 3 pairs by landing the three
attention rows on PSUM partitions {0,32,64} via column-embedded Wat
stationaries.

Per edge-row i the math is
  pre      = wc (x) d2[i,:] + hb + ha[:,i]     (K=1 fp32 matmul + DVE + ScalarE)
  m        = relu(We2^T relu(pre) + be2)       (bf16 matmul + ScalarE)
  att      = Wat^T m + madj[i,:] + bat         (bf16 matmuls; madj = -1e9 on
                                                non-edges => sigmoid == adj)
  w        = sigmoid(att)                      (ScalarE, batched over 6 rows)
  agg[:,i] = sum_j (m/100) * bcast(w)          (ones-matmul + DVE
                                                scalar_tensor_tensor fused)
"""

import os
import sys

import numpy as np

if not any(os.path.isdir(os.path.join(p, "concourse")) for p in sys.path if p):
    sys.path.insert(0, "/opt/trn_rl_repo")

# ---- problem constants (hardcoded per contest rules) ----
B, NS, NP = 8, 32, 224
N = NS + NP                      # 256 nodes
LIG_NF, POK_NF, JNF, HID, OUT_NF, NLAYERS = 10, 25, 32, 128, 128, 4
CUT2 = 4.5 ** 2
NORM = 100.0

_F32 = np.float32
NPAIR = N // 2                   # 128 row-pairs
_NB = (NPAIR + 2) // 3           # pair slots per base partition (43)


def _np_silu(x):
    return x / (1.0 + np.exp(-x))


def _host_prep(inputs):
    """Host-side preprocessing: embedding h0, pairwise d2, adjacency logits."""
    x = np.concatenate([inputs["mol_x"], inputs["pocket_x"]], axis=1).astype(_F32)
    mask = np.concatenate([inputs["node_mask"], inputs["pocket_mask"]], axis=1).astype(
        _F32
    )
    hm = _np_silu(inputs["mol_h"].astype(_F32) @ inputs["W_mol"] + inputs["b_mol"])
    hp = _np_silu(
        inputs["pocket_h"].astype(_F32) @ inputs["W_pok"] + inputs["b_pok"]
    )
    h0 = (
        np.concatenate([hm, hp], axis=1) @ inputs["W_emb"] + inputs["b_emb"]
    ).astype(_F32)  # [B, N, H]

    diff = x[:, :, None, :] - x[:, None, :, :]
    d2 = np.sum(diff * diff, axis=-1, dtype=_F32)  # [B, N, N]
    idx = np.arange(N)
    lig_pair = (idx[:, None] < NS) & (idx[None, :] < NS)
    adj = np.where(lig_pair, 1.0, (d2 <= CUT2).astype(_F32))
    adj = adj * mask[:, :, None] * mask[:, None, :]
    return h0, d2, adj, mask


def _pack_pairs3(mat, dtype):
    """[256, 256] -> [65, 43*512]: row pair (2t, 2t+1) at partition 32*(t%3),
    cols (t//3)*512. Matmul rhs APs must start at partition 0/32/64."""
    out = np.zeros((65, _NB * 2 * N), dtype=dtype)
    for t in range(NPAIR):
        c = (t // 3) * 2 * N
        out[32 * (t % 3), c : c + N] = mat[2 * t]
        out[32 * (t % 3), c + N : c + 2 * N] = mat[2 * t + 1]
    return out


def _pack_d2_split(d2):
    """bf16 hi/lo split of d2 rows: partition b holds d2_hi, b+1 d2_lo,
    b+2 d2_hi again (pairs with lhsT rows [wc_hi, wc_hi, wc_lo])."""
    import ml_dtypes

    bf = ml_dtypes.bfloat16
    out = np.zeros((67, _NB * 2 * N), dtype=bf)
    for t in range(NPAIR):
        b, c = 32 * (t % 3), (t // 3) * 2 * N
        row = np.concatenate([d2[2 * t], d2[2 * t + 1]]).astype(_F32)
        hi = row.astype(bf)
        lo = (row - hi.astype(_F32)).astype(bf)
        out[b, c : c + 2 * N] = hi
        out[b + 1, c : c + 2 * N] = lo
        out[b + 2, c : c + 2 * N] = hi
    return out


def _pack_wc_split(wcr):
    """[1, L*H] f32 -> [67, L*H] bf16 with rows [wc_hi, wc_hi, wc_lo] at each
    base partition in {0, 32, 64}."""
    import ml_dtypes

    bf = ml_dtypes.bfloat16
    hi = wcr.astype(bf)
    lo = (wcr.astype(_F32) - hi.astype(_F32)).astype(bf)
    out = np.zeros((67, wcr.shape[-1]), dtype=bf)
    for b in (0, 32, 64):
        out[b] = hi
        out[b + 1] = hi
        out[b + 2] = lo
    return out


def _rep3(row):
    out = np.zeros((65, row.shape[-1]), dtype=row.dtype)
    out[0] = out[32] = out[64] = row
    return out


# weight-pack column offsets (per layer stride)
_PL = 6 * HID + 8  # wa, wb, We2, Wn1a, Wn1b, Wn2 (128 each) + small cols
_W_COLS = NLAYERS * _PL + HID + 8  # + W_out + wlin/bout/blin
_PLB = HID + 3 * HID  # bf16 pack per layer: We2 | WatEmb x3 (M=128)
_WB_COLS = NLAYERS * _PLB


def _pack_weights(inputs):
    import ml_dtypes

    wp = np.zeros((HID, _W_COLS), dtype=_F32)
    wcr = np.zeros((1, NLAYERS * HID), dtype=_F32)
    wpb = np.zeros((HID, _WB_COLS), dtype=ml_dtypes.bfloat16)
    We1 = inputs["We1"].astype(_F32)
    for l in range(NLAYERS):
        o = l * _PL
        wp[:, o : o + HID] = We1[l, :HID, :]              # wa
        wp[:, o + HID : o + 2 * HID] = We1[l, HID : 2 * HID, :]  # wb
        wp[:, o + 2 * HID : o + 3 * HID] = inputs["We2"][l]
        wp[:, o + 3 * HID : o + 4 * HID] = inputs["Wn1"][l][:HID, :]
        wp[:, o + 4 * HID : o + 5 * HID] = inputs["Wn1"][l][HID:, :]
        wp[:, o + 5 * HID : o + 6 * HID] = inputs["Wn2"][l]
        c = o + 6 * HID
        wp[:, c + 1] = inputs["be1"][l]
        wp[:, c + 2] = inputs["be2"][l]
        wp[:, c + 3] = inputs["bn1"][l]
        wp[:, c + 4] = inputs["bn2"][l]
        wp[:, c + 5] = inputs["bat"][l][0]                # bat replicated
        wcr[0, l * HID : (l + 1) * HID] = We1[l, 2 * HID, :]
        ob = l * _PLB
        wpb[:, ob : ob + HID] = inputs["We2"][l]
        for k in range(3):
            wpb[:, ob + HID + HID * k + 32 * k] = inputs["Wat"][l][:, 0]
    o = NLAYERS * _PL
    wp[:, o : o + HID] = inputs["W_out"].astype(_F32)
    wp[:, o + HID] = inputs["W_lin"][:, 0]
    wp[:, o + HID + 1] = inputs["b_out"]
    wp[0, o + HID + 2] = inputs["b_lin"][0]
    return wp, wcr, wpb


def _build(nc, tile_mod, bass_mod, n_layers, n_i):
    """Trace the per-core kernel into nc (a Bacc)."""
    mybir = __import__("concourse.mybir", fromlist=["mybir"])
    dt = mybir.dt.float32
    bf = mybir.dt.bfloat16
    AF = mybir.ActivationFunctionType
    ALU = mybir.AluOpType
    N2 = 2 * N

    assert n_i % 2 == 0
    npair = n_i // 2

    hT_d = nc.dram_tensor("hT0", [HID, N], dt, kind="ExternalInput")
    d2_d = nc.dram_tensor("d2p", [67, _NB * N2], bf, kind="ExternalInput")
    ma_d = nc.dram_tensor("adjp", [65, _NB * N2], bf, kind="ExternalInput")
    wp_d = nc.dram_tensor("wpack", [HID, _W_COLS], dt, kind="ExternalInput")
    wc_d = nc.dram_tensor("wcrows", [67, NLAYERS * HID], bf, kind="ExternalInput")
    wb_d = nc.dram_tensor("wpackb", [HID, _WB_COLS], bf, kind="ExternalInput")
    out_d = nc.dram_tensor("out", [1, NS], dt, kind="ExternalOutput")

    with tile_mod.TileContext(nc) as tc:
        with (
            tc.tile_pool(name="const", bufs=1) as cpool,
            tc.tile_pool(name="layer", bufs=2) as lpool,
            tc.tile_pool(name="work", bufs=4) as wpool,
            tc.tile_pool(name="psA", bufs=3, space="PSUM") as psA,
            tc.tile_pool(name="psB", bufs=3, space="PSUM") as psB,
            tc.tile_pool(name="psC", bufs=1, space="PSUM") as psC,
            tc.tile_pool(name="psD", bufs=1, space="PSUM") as psD,
        ):
            # ---- load constants ----
            hT = cpool.tile([HID, N], dt, tag="hT0")
            d2p = cpool.tile([67, _NB * N2], bf, tag="d2p")
            adjp = cpool.tile([65, _NB * N2], bf, tag="adjp")
            wp = cpool.tile([HID, _W_COLS], dt, tag="wpack")
            wcr = cpool.tile([67, NLAYERS * HID], bf, tag="wcrows")
            wpb = cpool.tile([HID, _WB_COLS], bf, tag="wpackb")
            nc.sync.dma_start(hT[:], hT_d.ap())
            nc.sync.dma_start(d2p[:], d2_d.ap())
            nc.sync.dma_start(adjp[:], ma_d.ap())
            nc.sync.dma_start(wp[:], wp_d.ap())
            nc.sync.dma_start(wcr[:], wc_d.ap())
            nc.sync.dma_start(wpb[:], wb_d.ap())
            ones1_65 = cpool.tile([65, HID], bf, tag="ones1")
            nc.vector.memset(ones1_65[:], 1.0)

            hT_cur = hT
            for l in range(n_layers):
                o = l * _PL
                wa = wp[:, o : o + HID]
                wb = wp[:, o + HID : o + 2 * HID]
                Wn1a = wp[:, o + 3 * HID : o + 4 * HID]
                Wn1b = wp[:, o + 4 * HID : o + 5 * HID]
                Wn2 = wp[:, o + 5 * HID : o + 6 * HID]
                c = o + 6 * HID
                be1 = wp[:, c + 1 : c + 2]
                be2 = wp[:, c + 2 : c + 3]
                bn1 = wp[:, c + 3 : c + 4]
                bn2 = wp[:, c + 4 : c + 5]
                bat65 = wp[0:65, c + 5 : c + 6]
                u_last = -1
                ob = l * _PLB
                We2b = wpb[:, ob : ob + HID]

                # ---- per-layer node projections ----
                ps_ha = psA.tile([HID, N], dt, tag="pre")
                nc.tensor.matmul(ps_ha[:], wa, hT_cur[:], start=True, stop=True)
                haT = lpool.tile([HID, N], dt, tag="haT")
                nc.scalar.activation(haT[:], ps_ha[:], AF.Identity, bias=be1)
                ps_hb = psA.tile([HID, N], dt, tag="pre")
                nc.tensor.matmul(ps_hb[:], wb, hT_cur[:], start=True, stop=True)
                hbT2 = lpool.tile([HID, N2], dt, tag="hbT2")
                nc.vector.tensor_copy(hbT2[:, 0:N], ps_hb[:])
                nc.vector.tensor_copy(hbT2[:, N:N2], ps_hb[:])

                aggT = lpool.tile([HID, N], dt, tag="aggT")
                if n_i < N:
                    nc.vector.memset(aggT[:], 0.0)

                # ---- edge rows: groups of 3 pairs, stage-ordered; the
                # gating/aggregation tail of each group is emitted one group
                # late so the PE never stalls on the end of a group's chain.
                pending = None
                group_starts = list(range(0, npair, 3)) + [None]
                for t0 in group_starts:
                    if t0 is None:
                        ts = []
                    else:
                        ts = [t for t in range(t0, min(t0 + 3, npair))]
                    bs = [32 * (t % 3) for t in ts]
                    cs = [(t // 3) * N2 for t in ts]

                    ps_pre, pre2, rpre, ps_m1, m = {}, {}, {}, {}, {}
                    for k, t in enumerate(ts):
                        b, cc = bs[k], cs[k]
                        wc3 = wcr[b : b + 3, l * HID : (l + 1) * HID]
                        ps_pre[k] = psA.tile([HID, N2], dt, tag="pre", name=f"pspre{k}")
                        nc.tensor.matmul(
                            ps_pre[k][:],
                            wc3,
                            d2p[b : b + 3, cc : cc + N2],
                            start=True,
                            stop=True,
                        )
                    for k, t in enumerate(ts):
                        pre2[k] = wpool.tile([HID, N2], dt, tag="pre2", name=f"pre2_{k}")
                        for h in range(2):
                            i = 2 * t + h
                            nc.vector.scalar_tensor_tensor(
                                out=pre2[k][:, h * N : (h + 1) * N],
                                in0=ps_pre[k][:, h * N : (h + 1) * N],
                                scalar=haT[:, i : i + 1],
                                in1=hbT2[:, 0:N],
                                op0=ALU.add,
                                op1=ALU.add,
                            )
                    for k, t in enumerate(ts):
                        rpre[k] = wpool.tile([HID, N2], bf, tag="rpre", name=f"rpre{k}")
                        nc.scalar.activation(
                            rpre[k][:], pre2[k][:], AF.Relu, bias=0.0
                        )
                    for k, t in enumerate(ts):
                        ps_m1[k] = psB.tile([HID, N2], dt, tag="m1", name=f"psm1_{k}")
                        nc.tensor.matmul(
                            ps_m1[k][:], We2b, rpre[k][:], start=True, stop=True
                        )
                    for k, t in enumerate(ts):
                        m[k] = wpool.tile([HID, N2], bf, tag="m", name=f"m{k}", bufs=8)
                        nc.scalar.activation(
                            m[k][:], ps_m1[k][:], AF.Relu, bias=be2
                        )
                    if not ts:
                        sig3m = None
                    # attention rows of the 3 pairs land on partitions 0/32/64
                    if ts:
                        ps_att = psC.tile([128, N2], dt, tag="att")
                        for k, t in enumerate(ts):
                            WatE = wpb[
                                :, ob + HID + HID * k : ob + HID + HID * (k + 1)
                            ]
                            nc.tensor.matmul(
                                ps_att[:],
                                WatE,
                                m[k][:],
                                start=(k == 0),
                                stop=(k == len(ts) - 1),
                            )
                        sig3 = wpool.tile([65, N2], bf, tag="sig3")
                        nc.scalar.activation(
                            sig3[:], ps_att[0:65, :], AF.Sigmoid, bias=bat65
                        )
                        u = ts[0] // 3
                        sig3m = wpool.tile([65, N2], bf, tag="sig3m")
                        nc.vector.tensor_tensor(
                            sig3m[:],
                            sig3[:],
                            adjp[:, u * N2 : (u + 1) * N2],
                            ALU.mult,
                        )
                    if pending is not None:
                        p_ts, p_bs, p_m, p_sig3m = pending
                        for k, t in enumerate(p_ts):
                            b = p_bs[k]
                            ps_w = psD.tile([HID, N2], dt, tag="w")
                            nc.tensor.matmul(
                                ps_w[:],
                                ones1_65[b : b + 1, :],
                                p_sig3m[b : b + 1, :],
                                start=True,
                                stop=True,
                            )
                            for h in range(2):
                                i = 2 * t + h
                                mg = wpool.tile([HID, N], dt, tag="mg")
                                nc.vector.scalar_tensor_tensor(
                                    out=mg[:],
                                    in0=p_m[k][:, h * N : (h + 1) * N],
                                    scalar=1.0 / NORM,
                                    in1=ps_w[:, h * N : (h + 1) * N],
                                    op0=ALU.mult,
                                    op1=ALU.mult,
                                    accum_out=aggT[:, i : i + 1],
                                )
                    pending = (ts, bs, m, sig3m) if ts else None

                # ---- node MLP:  h += relu([h, agg] @ Wn1 + bn1) @ Wn2 + bn2 ----
                ps_n1 = psA.tile([HID, N], dt, tag="pre")
                nc.tensor.matmul(ps_n1[:], Wn1a, hT_cur[:], start=True, stop=False)
                nc.tensor.matmul(ps_n1[:], Wn1b, aggT[:], start=False, stop=True)
                t1 = wpool.tile([HID, N], dt, tag="pre2")
                nc.scalar.activation(t1[:], ps_n1[:], AF.Relu, bias=bn1)
                ps_n2 = psB.tile([HID, N], dt, tag="m1")
                nc.tensor.matmul(ps_n2[:], Wn2, t1[:], start=True, stop=True)
                hsum = wpool.tile([HID, N], dt, tag="pre2")
                nc.vector.tensor_tensor(hsum[:], ps_n2[:], hT_cur[:], ALU.add)
                hT_new = lpool.tile([HID, N], dt, tag="hT")
                nc.scalar.activation(hT_new[:], hsum[:], AF.Identity, bias=bn2)
                hT_cur = hT_new

            # ---- output head ----
            o = NLAYERS * _PL
            W_out = wp[:, o : o + HID]
            W_lin = wp[:, o + HID : o + HID + 1]
            b_out = wp[:, o + HID + 1 : o + HID + 2]
            b_lin = wp[0:1, o + HID + 2 : o + HID + 3]
            ps_o = psA.tile([HID, NS], dt, tag="pre")
            nc.tensor.matmul(ps_o[:], W_out, hT_cur[:, 0:NS], start=True, stop=True)
            ho = wpool.tile([HID, NS], dt, tag="pre2")
            nc.scalar.activation(ho[:], ps_o[:], AF.Relu, bias=b_out)
            ps_y = psC.tile([1, NS], dt, tag="att")
            nc.tensor.matmul(ps_y[:], W_lin, ho[:], start=True, stop=True)
            y = wpool.tile([1, NS], dt, tag="sig3")
            nc.scalar.activation(y[:], ps_y[:], AF.Identity, bias=b_lin)
            nc.sync.dma_start(out_d.ap(), y[:])


def _make_in_maps(inputs, n_layers, n_i):
    import ml_dtypes

    h0, d2, adj, mask = _host_prep(inputs)
    wp, wcr, wpb = _pack_weights(inputs)
    wcr3 = _pack_wc_split(wcr)
    in_maps = []
    for b in range(B):
        in_maps.append(
            {
                "hT0": np.ascontiguousarray(h0[b].T),
                "d2p": _pack_d2_split(d2[b]),
                "adjp": _pack_pairs3(adj[b], ml_dtypes.bfloat16),
                "wpack": wp,
                "wcrows": wcr3,
                "wpackb": wpb,
            }
        )
    return in_maps, mask


def _install_ntff_hook():
    """Recreate the antenv.axon_hooks module the boot expected, register the
    ctypes NTFF hook from trn_agent_boot, so run_bass_kernel_spmd(trace=True)
    can capture hardware profiles under axon."""
    import types

    if "antenv.axon_hooks" not in sys.modules:
        mod = types.ModuleType("antenv.axon_hooks")
        holder = [None]
        mod.set_axon_ntff_profile_hook = lambda h: holder.__setitem__(0, h)
        mod.get_axon_ntff_profile_hook = lambda: holder[0]
        sys.modules["antenv.axon_hooks"] = mod
        import antenv

        antenv.axon_hooks = mod
    m = sys.modules["antenv.axon_hooks"]
    if m.get_axon_ntff_profile_hook() is None:
        sys.path.insert(0, "/root/.axon_site")
        from trn_agent_boot.trn_boot import _ntff_profile_via_ctypes

        m.set_axon_ntff_profile_hook(
            _ntff_profile_via_ctypes("/opt/axon/libaxon_pjrt.so")
        )


_CACHE = {}


def _get_nc(n_layers, n_i):
    key = (n_layers, n_i)
    if key not in _CACHE:
        import concourse.bass as bass
        import concourse.tile as tile
        from concourse import bacc

        nc = bacc.Bacc(
            "TRN2", target_bir_lowering=False, debug=False, num_devices=B
        )
        _build(nc, tile, bass, n_layers, n_i)
        nc.compile()
        _CACHE[key] = nc
    return _CACHE[key]


def kernel(**inputs):
    n_layers = int(os.environ.get("GNN_LAYERS", NLAYERS))
    n_i = int(os.environ.get("GNN_NI", N))
    in_maps, mask = _make_in_maps(inputs, n_layers, n_i)
    nc = _get_nc(n_layers, n_i)

    if os.environ.get("GNN_SIM"):
        from concourse.bass_interp import CoreSim

        sim = CoreSim(nc, trace=False)
        outs = []
        for b in range(int(os.environ.get("GNN_SIM_CORES", 1))):
            for k, v in in_maps[b].items():
                sim.tensor(k)[:] = v
            sim.simulate()
            outs.append(np.array(sim.tensor("out")).reshape(NS, 1))
        while len(outs) < B:
            outs.append(np.zeros((NS, 1), _F32))
        out = np.stack(outs)
    else:
        from concourse.bass_utils import run_bass_kernel_spmd

        if os.environ.get("GNN_TRACE"):
            _install_ntff_hook()
            tmpdir = os.environ.get("GNN_TRACE_DIR") or None
            try:
                res = run_bass_kernel_spmd(
                    nc, in_maps, core_ids=list(range(B)), trace=True, tmpdir=tmpdir
                )
                kernel.last_exec_time_ns = res.exec_time_ns
            except Exception as e:
                print(f"[gnn] traced run failed ({e!r}); retrying untraced")
                res = run_bass_kernel_spmd(nc, in_maps, core_ids=list(range(B)))
        else:
            res = run_bass_kernel_spmd(nc, in_maps, core_ids=list(range(B)))
        kernel.last_results = res
        out = np.stack([r["out"].reshape(NS, 1) for r in res.results])

    return (out * inputs["node_mask"][:, :, None]).astype(_F32)


# revision 21
# speedup vs baseline: 1.1849x; 1.1849x over previous
"""Trainium2 Bass kernel for AnchorGNNPocket (GNN message passing).

Data-parallel over batch B=8: one complex per NeuronCore. Each core runs the
full 4-layer GCL stack on its sample.

v2: edge rows processed in PAIRS ([128, 512] tiles = one PSUM bank), emitted
stage-by-stage over groups of 3 pairs (6 rows) so each engine sees batches of
independent work (software pipelining); edge-MLP matmuls in bf16 (fp32 matmul
is 2-pass LOW_HIGH on trn2); sigmoid batched over 3 pairs by landing the three
attention rows on PSUM partitions {0,32,64} via column-embedded Wat
stationaries.

Per edge-row i the math is
  pre      = wc (x) d2[i,:] + hb + ha[:,i]     (K=1 fp32 matmul + DVE + ScalarE)
  m        = relu(We2^T relu(pre) + be2)       (bf16 matmul + ScalarE)
  att      = Wat^T m + madj[i,:] + bat         (bf16 matmuls; madj = -1e9 on
                                                non-edges => sigmoid == adj)
  w        = sigmoid(att)                      (ScalarE, batched over 6 rows)
  agg[:,i] = sum_j (m/100) * bcast(w)          (ones-matmul + DVE
                                                scalar_tensor_tensor fused)
"""

import os
import sys

import numpy as np

if not any(os.path.isdir(os.path.join(p, "concourse")) for p in sys.path if p):
    sys.path.insert(0, "/opt/trn_rl_repo")

# ---- problem constants (hardcoded per contest rules) ----
B, NS, NP = 8, 32, 224
N = NS + NP                      # 256 nodes
LIG_NF, POK_NF, JNF, HID, OUT_NF, NLAYERS = 10, 25, 32, 128, 128, 4
CUT2 = 4.5 ** 2
NORM = 100.0

_F32 = np.float32
NPAIR = N // 2                   # 128 row-pairs
_NB = (NPAIR + 2) // 3           # pair slots per base partition (43)


def _np_silu(x):
    return x / (1.0 + np.exp(-x))


def _host_prep(inputs):
    """Host-side preprocessing: embedding h0, pairwise d2, adjacency logits."""
    x = np.concatenate([inputs["mol_x"], inputs["pocket_x"]], axis=1).astype(_F32)
    mask = np.concatenate([inputs["node_mask"], inputs["pocket_mask"]], axis=1).astype(
        _F32
    )
    hm = _np_silu(inputs["mol_h"].astype(_F32) @ inputs["W_mol"] + inputs["b_mol"])
    hp = _np_silu(
        inputs["pocket_h"].astype(_F32) @ inputs["W_pok"] + inputs["b_pok"]
    )
    h0 = (
        np.concatenate([hm, hp], axis=1) @ inputs["W_emb"] + inputs["b_emb"]
    ).astype(_F32)  # [B, N, H]

    diff = x[:, :, None, :] - x[:, None, :, :]
    d2 = np.sum(diff * diff, axis=-1, dtype=_F32)  # [B, N, N]
    idx = np.arange(N)
    lig_pair = (idx[:, None] < NS) & (idx[None, :] < NS)
    adj = np.where(lig_pair, 1.0, (d2 <= CUT2).astype(_F32))
    adj = adj * mask[:, :, None] * mask[:, None, :]
    return h0, d2, adj, mask


def _pack_pairs3(mat, dtype):
    """[256, 256] -> [65, 43*512]: row pair (2t, 2t+1) at partition 32*(t%3),
    cols (t//3)*512. Matmul rhs APs must start at partition 0/32/64."""
    out = np.zeros((65, _NB * 2 * N), dtype=dtype)
    for t in range(NPAIR):
        c = (t // 3) * 2 * N
        out[32 * (t % 3), c : c + N] = mat[2 * t]
        out[32 * (t % 3), c + N : c + 2 * N] = mat[2 * t + 1]
    return out


def _pack_d2_split(d2):
    """bf16 hi/lo split of d2 rows: partition b holds d2_hi, b+1 d2_lo,
    b+2 d2_hi again (pairs with lhsT rows [wc_hi, wc_hi, wc_lo])."""
    import ml_dtypes

    bf = ml_dtypes.bfloat16
    out = np.zeros((67, _NB * 2 * N), dtype=bf)
    for t in range(NPAIR):
        b, c = 32 * (t % 3), (t // 3) * 2 * N
        row = np.concatenate([d2[2 * t], d2[2 * t + 1]]).astype(_F32)
        hi = row.astype(bf)
        lo = (row - hi.astype(_F32)).astype(bf)
        out[b, c : c + 2 * N] = hi
        out[b + 1, c : c + 2 * N] = lo
        out[b + 2, c : c + 2 * N] = hi
    return out


def _pack_wc_split(wcr):
    """[1, L*H] f32 -> [67, L*H] bf16 with rows [wc_hi, wc_hi, wc_lo] at each
    base partition in {0, 32, 64}."""
    import ml_dtypes

    bf = ml_dtypes.bfloat16
    hi = wcr.astype(bf)
    lo = (wcr.astype(_F32) - hi.astype(_F32)).astype(bf)
    out = np.zeros((67, wcr.shape[-1]), dtype=bf)
    for b in (0, 32, 64):
        out[b] = hi
        out[b + 1] = hi
        out[b + 2] = lo
    return out


def _rep3(row):
    out = np.zeros((65, row.shape[-1]), dtype=row.dtype)
    out[0] = out[32] = out[64] = row
    return out


# weight-pack column offsets (per layer stride)
_PL = 6 * HID + 8  # wa, wb, We2, Wn1a, Wn1b, Wn2 (128 each) + small cols
_W_COLS = NLAYERS * _PL + HID + 8  # + W_out + wlin/bout/blin
_PLB = HID + 3 * HID  # bf16 pack per layer: We2 | WatEmb x3 (M=128)
_WB_COLS = NLAYERS * _PLB


def _pack_weights(inputs):
    import ml_dtypes

    wp = np.zeros((HID, _W_COLS), dtype=_F32)
    wcr = np.zeros((1, NLAYERS * HID), dtype=_F32)
    wpb = np.zeros((HID, _WB_COLS), dtype=ml_dtypes.bfloat16)
    We1 = inputs["We1"].astype(_F32)
    for l in range(NLAYERS):
        o = l * _PL
        wp[:, o : o + HID] = We1[l, :HID, :]              # wa
        wp[:, o + HID : o + 2 * HID] = We1[l, HID : 2 * HID, :]  # wb
        wp[:, o + 2 * HID : o + 3 * HID] = inputs["We2"][l]
        wp[:, o + 3 * HID : o + 4 * HID] = inputs["Wn1"][l][:HID, :]
        wp[:, o + 4 * HID : o + 5 * HID] = inputs["Wn1"][l][HID:, :]
        wp[:, o + 5 * HID : o + 6 * HID] = inputs["Wn2"][l]
        c = o + 6 * HID
        wp[:, c + 1] = inputs["be1"][l]
        wp[:, c + 2] = inputs["be2"][l]
        wp[:, c + 3] = inputs["bn1"][l]
        wp[:, c + 4] = inputs["bn2"][l]
        wp[:, c + 5] = inputs["bat"][l][0]                # bat replicated
        wcr[0, l * HID : (l + 1) * HID] = We1[l, 2 * HID, :]
        ob = l * _PLB
        wpb[:, ob : ob + HID] = inputs["We2"][l]
        for k in range(3):
            wpb[:, ob + HID + HID * k + 32 * k] = inputs["Wat"][l][:, 0]
    o = NLAYERS * _PL
    wp[:, o : o + HID] = inputs["W_out"].astype(_F32)
    wp[:, o + HID] = inputs["W_lin"][:, 0]
    wp[:, o + HID + 1] = inputs["b_out"]
    wp[0, o + HID + 2] = inputs["b_lin"][0]
    return wp, wcr, wpb


def _build(nc, tile_mod, bass_mod, n_layers, n_i):
    """Trace the per-core kernel into nc (a Bacc)."""
    mybir = __import__("concourse.mybir", fromlist=["mybir"])
    dt = mybir.dt.float32
    bf = mybir.dt.bfloat16
    AF = mybir.ActivationFunctionType
    ALU = mybir.AluOpType
    N2 = 2 * N

    assert n_i % 2 == 0
    npair = n_i // 2

    hT_d = nc.dram_tensor("hT0", [HID, N], dt, kind="ExternalInput")
    d2_d = nc.dram_tensor("d2p", [67, _NB * N2], bf, kind="ExternalInput")
    ma_d = nc.dram_tensor("adjp", [65, _NB * N2], bf, kind="ExternalInput")
    wp_d = nc.dram_tensor("wpack", [HID, _W_COLS], dt, kind="ExternalInput")
    wc_d = nc.dram_tensor("wcrows", [67, NLAYERS * HID], bf, kind="ExternalInput")
    wb_d = nc.dram_tensor("wpackb", [HID, _WB_COLS], bf, kind="ExternalInput")
    out_d = nc.dram_tensor("out", [1, NS], dt, kind="ExternalOutput")

    with tile_mod.TileContext(nc) as tc:
        with (
            tc.tile_pool(name="const", bufs=1) as cpool,
            tc.tile_pool(name="layer", bufs=2) as lpool,
            tc.tile_pool(name="work", bufs=4) as wpool,
            tc.tile_pool(name="psA", bufs=3, space="PSUM") as psA,
            tc.tile_pool(name="psB", bufs=3, space="PSUM") as psB,
            tc.tile_pool(name="psC", bufs=1, space="PSUM") as psC,
            tc.tile_pool(name="psD", bufs=1, space="PSUM") as psD,
        ):
            # ---- load constants ----
            hT = cpool.tile([HID, N], dt, tag="hT0")
            d2p = cpool.tile([67, _NB * N2], bf, tag="d2p")
            adjp = cpool.tile([65, _NB * N2], bf, tag="adjp")
            wp = cpool.tile([HID, _W_COLS], dt, tag="wpack")
            wcr = cpool.tile([67, NLAYERS * HID], bf, tag="wcrows")
            wpb = cpool.tile([HID, _WB_COLS], bf, tag="wpackb")
            nc.sync.dma_start(hT[:], hT_d.ap())
            nc.sync.dma_start(d2p[:], d2_d.ap())
            nc.sync.dma_start(adjp[:], ma_d.ap())
            nc.sync.dma_start(wp[:], wp_d.ap())
            nc.sync.dma_start(wcr[:], wc_d.ap())
            nc.sync.dma_start(wpb[:], wb_d.ap())
            ones1_65 = cpool.tile([65, HID], bf, tag="ones1")
            nc.vector.memset(ones1_65[:], 1.0)

            hT_cur = hT
            for l in range(n_layers):
                o = l * _PL
                wa = wp[:, o : o + HID]
                wb = wp[:, o + HID : o + 2 * HID]
                Wn1a = wp[:, o + 3 * HID : o + 4 * HID]
                Wn1b = wp[:, o + 4 * HID : o + 5 * HID]
                Wn2 = wp[:, o + 5 * HID : o + 6 * HID]
                c = o + 6 * HID
                be1 = wp[:, c + 1 : c + 2]
                be2 = wp[:, c + 2 : c + 3]
                bn1 = wp[:, c + 3 : c + 4]
                bn2 = wp[:, c + 4 : c + 5]
                bat65 = wp[0:65, c + 5 : c + 6]
                u_last = -1
                ob = l * _PLB
                We2b = wpb[:, ob : ob + HID]

                # ---- per-layer node projections ----
                ps_ha = psA.tile([HID, N], dt, tag="pre")
                nc.tensor.matmul(ps_ha[:], wa, hT_cur[:], start=True, stop=True)
                haT = lpool.tile([HID, N], dt, tag="haT")
                nc.scalar.activation(haT[:], ps_ha[:], AF.Identity, bias=be1)
                ps_hb = psA.tile([HID, N], dt, tag="pre")
                nc.tensor.matmul(ps_hb[:], wb, hT_cur[:], start=True, stop=True)
                hbT2 = lpool.tile([HID, N2], dt, tag="hbT2")
                nc.vector.tensor_copy(hbT2[:, 0:N], ps_hb[:])
                nc.vector.tensor_copy(hbT2[:, N:N2], ps_hb[:])

                aggT = lpool.tile([HID, N], dt, tag="aggT")
                if n_i < N:
                    nc.vector.memset(aggT[:], 0.0)

                # ---- edge rows: groups of 3 pairs, stage-ordered; the
                # gating/aggregation tail of each group is emitted one group
                # late so the PE never stalls on the end of a group's chain.
                pending = None
                group_starts = list(range(0, npair, 3)) + [None]
                for t0 in group_starts:
                    if t0 is None:
                        ts = []
                    else:
                        ts = [t for t in range(t0, min(t0 + 3, npair))]
                    bs = [32 * (t % 3) for t in ts]
                    cs = [(t // 3) * N2 for t in ts]

                    ps_pre, pre2, rpre, ps_m1, m = {}, {}, {}, {}, {}
                    for k, t in enumerate(ts):
                        b, cc = bs[k], cs[k]
                        wc3 = wcr[b : b + 3, l * HID : (l + 1) * HID]
                        ps_pre[k] = psA.tile([HID, N2], dt, tag="pre", name=f"pspre{k}")
                        nc.tensor.matmul(
                            ps_pre[k][:],
                            wc3,
                            d2p[b : b + 3, cc : cc + N2],
                            start=True,
                            stop=True,
                        )
                    for k, t in enumerate(ts):
                        pre2[k] = wpool.tile([HID, N2], dt, tag="pre2", name=f"pre2_{k}")
                        for h in range(2):
                            i = 2 * t + h
                            nc.vector.scalar_tensor_tensor(
                                out=pre2[k][:, h * N : (h + 1) * N],
                                in0=ps_pre[k][:, h * N : (h + 1) * N],
                                scalar=haT[:, i : i + 1],
                                in1=hbT2[:, 0:N],
                                op0=ALU.add,
                                op1=ALU.add,
                            )
                    for k, t in enumerate(ts):
                        rpre[k] = wpool.tile([HID, N2], bf, tag="rpre", name=f"rpre{k}")
                        nc.scalar.activation(
                            rpre[k][:], pre2[k][:], AF.Relu, bias=0.0
                        )
                    for k, t in enumerate(ts):
                        ps_m1[k] = psB.tile([HID, N2], dt, tag="m1", name=f"psm1_{k}")
                        nc.tensor.matmul(
                            ps_m1[k][:], We2b, rpre[k][:], start=True, stop=True
                        )
                    for k, t in enumerate(ts):
                        m[k] = wpool.tile([HID, N2], bf, tag="m", name=f"m{k}", bufs=8)
                        nc.scalar.activation(
                            m[k][:], ps_m1[k][:], AF.Relu, bias=be2
                        )
                    if not ts:
                        sig3m = None
                    # attention rows of the 3 pairs land on partitions 0/32/64
                    if ts:
                        ps_att = psC.tile([128, N2], dt, tag="att")
                        for k, t in enumerate(ts):
                            WatE = wpb[
                                :, ob + HID + HID * k : ob + HID + HID * (k + 1)
                            ]
                            nc.tensor.matmul(
                                ps_att[:],
                                WatE,
                                m[k][:],
                                start=(k == 0),
                                stop=(k == len(ts) - 1),
                            )
                        sig3 = wpool.tile([65, N2], bf, tag="sig3")
                        nc.scalar.activation(
                            sig3[:], ps_att[0:65, :], AF.Sigmoid, bias=bat65
                        )
                        u = ts[0] // 3
                        sig3m = wpool.tile([65, N2], bf, tag="sig3m")
                        nc.vector.tensor_tensor(
                            sig3m[:],
                            sig3[:],
                            adjp[:, u * N2 : (u + 1) * N2],
                            ALU.mult,
                        )
                    pending = (ts, bs, m, sig3m) if ts else None
                    if pending is not None:
                        p_ts, p_bs, p_m, p_sig3m = pending
                        for k, t in enumerate(p_ts):
                            b = p_bs[k]
                            ps_w = psD.tile([HID, N2], dt, tag="w")
                            nc.tensor.matmul(
                                ps_w[:],
                                ones1_65[b : b + 1, :],
                                p_sig3m[b : b + 1, :],
                                start=True,
                                stop=True,
                            )
                            for h in range(2):
                                i = 2 * t + h
                                mg = wpool.tile([HID, N], dt, tag="mg")
                                nc.vector.scalar_tensor_tensor(
                                    out=mg[:],
                                    in0=p_m[k][:, h * N : (h + 1) * N],
                                    scalar=1.0 / NORM,
                                    in1=ps_w[:, h * N : (h + 1) * N],
                                    op0=ALU.mult,
                                    op1=ALU.mult,
                                    accum_out=aggT[:, i : i + 1],
                                )

                # ---- node MLP:  h += relu([h, agg] @ Wn1 + bn1) @ Wn2 + bn2 ----
                ps_n1 = psA.tile([HID, N], dt, tag="pre")
                nc.tensor.matmul(ps_n1[:], Wn1a, hT_cur[:], start=True, stop=False)
                nc.tensor.matmul(ps_n1[:], Wn1b, aggT[:], start=False, stop=True)
                t1 = wpool.tile([HID, N], dt, tag="pre2")
                nc.scalar.activation(t1[:], ps_n1[:], AF.Relu, bias=bn1)
                ps_n2 = psB.tile([HID, N], dt, tag="m1")
                nc.tensor.matmul(ps_n2[:], Wn2, t1[:], start=True, stop=True)
                hsum = wpool.tile([HID, N], dt, tag="pre2")
                nc.vector.tensor_tensor(hsum[:], ps_n2[:], hT_cur[:], ALU.add)
                hT_new = lpool.tile([HID, N], dt, tag="hT")
                nc.scalar.activation(hT_new[:], hsum[:], AF.Identity, bias=bn2)
                hT_cur = hT_new

            # ---- output head ----
            o = NLAYERS * _PL
            W_out = wp[:, o : o + HID]
            W_lin = wp[:, o + HID : o + HID + 1]
            b_out = wp[:, o + HID + 1 : o + HID + 2]
            b_lin = wp[0:1, o + HID + 2 : o + HID + 3]
            ps_o = psA.tile([HID, NS], dt, tag="pre")
            nc.tensor.matmul(ps_o[:], W_out, hT_cur[:, 0:NS], start=True, stop=True)
            ho = wpool.tile([HID, NS], dt, tag="pre2")
            nc.scalar.activation(ho[:], ps_o[:], AF.Relu, bias=b_out)
            ps_y = psC.tile([1, NS], dt, tag="att")
            nc.tensor.matmul(ps_y[:], W_lin, ho[:], start=True, stop=True)
            y = wpool.tile([1, NS], dt, tag="sig3")
            nc.scalar.activation(y[:], ps_y[:], AF.Identity, bias=b_lin)
            nc.sync.dma_start(out_d.ap(), y[:])


def _make_in_maps(inputs, n_layers, n_i):
    import ml_dtypes

    h0, d2, adj, mask = _host_prep(inputs)
    wp, wcr, wpb = _pack_weights(inputs)
    wcr3 = _pack_wc_split(wcr)
    in_maps = []
    for b in range(B):
        in_maps.append(
            {
                "hT0": np.ascontiguousarray(h0[b].T),
                "d2p": _pack_d2_split(d2[b]),
                "adjp": _pack_pairs3(adj[b], ml_dtypes.bfloat16),
                "wpack": wp,
                "wcrows": wcr3,
                "wpackb": wpb,
            }
        )
    return in_maps, mask


def _install_ntff_hook():
    """Recreate the antenv.axon_hooks module the boot expected, register the
    ctypes NTFF hook from trn_agent_boot, so run_bass_kernel_spmd(trace=True)
    can capture hardware profiles under axon."""
    import types

    if "antenv.axon_hooks" not in sys.modules:
        mod = types.ModuleType("antenv.axon_hooks")
        holder = [None]
        mod.set_axon_ntff_profile_hook = lambda h: holder.__setitem__(0, h)
        mod.get_axon_ntff_profile_hook = lambda: holder[0]
        sys.modules["antenv.axon_hooks"] = mod
        import antenv

        antenv.axon_hooks = mod
    m = sys.modules["antenv.axon_hooks"]
    if m.get_axon_ntff_profile_hook() is None:
        sys.path.insert(0, "/root/.axon_site")
        from trn_agent_boot.trn_boot import _ntff_profile_via_ctypes

        m.set_axon_ntff_profile_hook(
            _ntff_profile_via_ctypes("/opt/axon/libaxon_pjrt.so")
        )


_CACHE = {}


def _get_nc(n_layers, n_i):
    key = (n_layers, n_i)
    if key not in _CACHE:
        import concourse.bass as bass
        import concourse.tile as tile
        from concourse import bacc

        nc = bacc.Bacc(
            "TRN2", target_bir_lowering=False, debug=False, num_devices=B
        )
        _build(nc, tile, bass, n_layers, n_i)
        nc.compile()
        _CACHE[key] = nc
    return _CACHE[key]


def kernel(**inputs):
    n_layers = int(os.environ.get("GNN_LAYERS", NLAYERS))
    n_i = int(os.environ.get("GNN_NI", N))
    in_maps, mask = _make_in_maps(inputs, n_layers, n_i)
    nc = _get_nc(n_layers, n_i)

    if os.environ.get("GNN_SIM"):
        from concourse.bass_interp import CoreSim

        sim = CoreSim(nc, trace=False)
        outs = []
        for b in range(int(os.environ.get("GNN_SIM_CORES", 1))):
            for k, v in in_maps[b].items():
                sim.tensor(k)[:] = v
            sim.simulate()
            outs.append(np.array(sim.tensor("out")).reshape(NS, 1))
        while len(outs) < B:
            outs.append(np.zeros((NS, 1), _F32))
        out = np.stack(outs)
    else:
        from concourse.bass_utils import run_bass_kernel_spmd

        if os.environ.get("GNN_TRACE"):
            _install_ntff_hook()
            tmpdir = os.environ.get("GNN_TRACE_DIR") or None
            try:
                res = run_bass_kernel_spmd(
                    nc, in_maps, core_ids=list(range(B)), trace=True, tmpdir=tmpdir
                )
                kernel.last_exec_time_ns = res.exec_time_ns
            except Exception as e:
                print(f"[gnn] traced run failed ({e!r}); retrying untraced")
                res = run_bass_kernel_spmd(nc, in_maps, core_ids=list(range(B)))
        else:
            res = run_bass_kernel_spmd(nc, in_maps, core_ids=list(range(B)))
        kernel.last_results = res
        out = np.stack([r["out"].reshape(NS, 1) for r in res.results])

    return (out * inputs["node_mask"][:, :, None]).astype(_F32)


# revision 23
# speedup vs baseline: 1.2567x; 1.0606x over previous
"""Trainium2 Bass kernel for AnchorGNNPocket (GNN message passing).

Data-parallel over batch B=8: one complex per NeuronCore. Each core runs the
full 4-layer GCL stack on its sample.

v2: edge rows processed in PAIRS ([128, 512] tiles = one PSUM bank), emitted
stage-by-stage over groups of 3 pairs (6 rows) so each engine sees batches of
independent work (software pipelining); edge-MLP matmuls in bf16 (fp32 matmul
is 2-pass LOW_HIGH on trn2); sigmoid batched over 3 pairs by landing the three
attention rows on PSUM partitions {0,32,64} via column-embedded Wat
stationaries.

Per edge-row i the math is
  pre      = wc (x) d2[i,:] + hb + ha[:,i]     (K=1 fp32 matmul + DVE + ScalarE)
  m        = relu(We2^T relu(pre) + be2)       (bf16 matmul + ScalarE)
  att      = Wat^T m + madj[i,:] + bat         (bf16 matmuls; madj = -1e9 on
                                                non-edges => sigmoid == adj)
  w        = sigmoid(att)                      (ScalarE, batched over 6 rows)
  agg[:,i] = sum_j (m/100) * bcast(w)          (ones-matmul + DVE
                                                scalar_tensor_tensor fused)
"""

import os
import sys

import numpy as np

if not any(os.path.isdir(os.path.join(p, "concourse")) for p in sys.path if p):
    sys.path.insert(0, "/opt/trn_rl_repo")

# ---- problem constants (hardcoded per contest rules) ----
B, NS, NP = 8, 32, 224
N = NS + NP                      # 256 nodes
LIG_NF, POK_NF, JNF, HID, OUT_NF, NLAYERS = 10, 25, 32, 128, 128, 4
CUT2 = 4.5 ** 2
NORM = 100.0

_F32 = np.float32
NPAIR = N // 2                   # 128 row-pairs
_NB = (NPAIR + 2) // 3           # pair slots per base partition (43)


def _np_silu(x):
    return x / (1.0 + np.exp(-x))


def _host_prep(inputs):
    """Host-side preprocessing: embedding h0, pairwise d2, adjacency logits."""
    x = np.concatenate([inputs["mol_x"], inputs["pocket_x"]], axis=1).astype(_F32)
    mask = np.concatenate([inputs["node_mask"], inputs["pocket_mask"]], axis=1).astype(
        _F32
    )
    hm = _np_silu(inputs["mol_h"].astype(_F32) @ inputs["W_mol"] + inputs["b_mol"])
    hp = _np_silu(
        inputs["pocket_h"].astype(_F32) @ inputs["W_pok"] + inputs["b_pok"]
    )
    h0 = (
        np.concatenate([hm, hp], axis=1) @ inputs["W_emb"] + inputs["b_emb"]
    ).astype(_F32)  # [B, N, H]

    diff = x[:, :, None, :] - x[:, None, :, :]
    d2 = np.sum(diff * diff, axis=-1, dtype=_F32)  # [B, N, N]
    idx = np.arange(N)
    lig_pair = (idx[:, None] < NS) & (idx[None, :] < NS)
    adj = np.where(lig_pair, 1.0, (d2 <= CUT2).astype(_F32))
    adj = adj * mask[:, :, None] * mask[:, None, :]
    madj = np.where(adj > 0, 0.0, -1.0e9).astype(_F32)
    return h0, d2, madj, mask


def _pack_pairs3(mat, dtype):
    """[256, 256] -> [65, 43*512]: row pair (2t, 2t+1) at partition 32*(t%3),
    cols (t//3)*512. Matmul rhs APs must start at partition 0/32/64."""
    out = np.zeros((65, _NB * 2 * N), dtype=dtype)
    for t in range(NPAIR):
        c = (t // 3) * 2 * N
        out[32 * (t % 3), c : c + N] = mat[2 * t]
        out[32 * (t % 3), c + N : c + 2 * N] = mat[2 * t + 1]
    return out


def _pack_d2_split(d2):
    """bf16 hi/lo split of d2 rows: partition b holds d2_hi, b+1 d2_lo,
    b+2 d2_hi again (pairs with lhsT rows [wc_hi, wc_hi, wc_lo])."""
    import ml_dtypes

    bf = ml_dtypes.bfloat16
    out = np.zeros((67, _NB * 2 * N), dtype=bf)
    for t in range(NPAIR):
        b, c = 32 * (t % 3), (t // 3) * 2 * N
        row = np.concatenate([d2[2 * t], d2[2 * t + 1]]).astype(_F32)
        hi = row.astype(bf)
        lo = (row - hi.astype(_F32)).astype(bf)
        out[b, c : c + 2 * N] = hi
        out[b + 1, c : c + 2 * N] = lo
        out[b + 2, c : c + 2 * N] = hi
    return out


def _pack_wc_split(wcr):
    """[1, L*H] f32 -> [67, L*H] bf16 with rows [wc_hi, wc_hi, wc_lo] at each
    base partition in {0, 32, 64}."""
    import ml_dtypes

    bf = ml_dtypes.bfloat16
    hi = wcr.astype(bf)
    lo = (wcr.astype(_F32) - hi.astype(_F32)).astype(bf)
    out = np.zeros((67, wcr.shape[-1]), dtype=bf)
    for b in (0, 32, 64):
        out[b] = hi
        out[b + 1] = hi
        out[b + 2] = lo
    return out


def _rep3(row):
    out = np.zeros((65, row.shape[-1]), dtype=row.dtype)
    out[0] = out[32] = out[64] = row
    return out


# weight-pack column offsets (per layer stride)
_PL = 6 * HID + 8  # wa, wb, We2, Wn1a, Wn1b, Wn2 (128 each) + small cols
_W_COLS = NLAYERS * _PL + HID + 8  # + W_out + wlin/bout/blin
_PLB = 2 * HID  # bf16 pack per layer: We2 | WatFull (Wat in all 128 cols)
_WB_COLS = NLAYERS * _PLB


def _pack_weights(inputs):
    import ml_dtypes

    wp = np.zeros((HID, _W_COLS), dtype=_F32)
    wcr = np.zeros((1, NLAYERS * HID), dtype=_F32)
    wpb = np.zeros((HID, _WB_COLS), dtype=ml_dtypes.bfloat16)
    We1 = inputs["We1"].astype(_F32)
    for l in range(NLAYERS):
        o = l * _PL
        wp[:, o : o + HID] = We1[l, :HID, :]              # wa
        wp[:, o + HID : o + 2 * HID] = We1[l, HID : 2 * HID, :]  # wb
        wp[:, o + 2 * HID : o + 3 * HID] = inputs["We2"][l]
        wp[:, o + 3 * HID : o + 4 * HID] = inputs["Wn1"][l][:HID, :]
        wp[:, o + 4 * HID : o + 5 * HID] = inputs["Wn1"][l][HID:, :]
        wp[:, o + 5 * HID : o + 6 * HID] = inputs["Wn2"][l]
        c = o + 6 * HID
        wp[:, c + 1] = inputs["be1"][l]
        wp[:, c + 2] = inputs["be2"][l]
        wp[:, c + 3] = inputs["bn1"][l]
        wp[:, c + 4] = inputs["bn2"][l]
        wp[:, c + 5] = inputs["bat"][l][0]                # bat replicated
        wcr[0, l * HID : (l + 1) * HID] = We1[l, 2 * HID, :]
        ob = l * _PLB
        wpb[:, ob : ob + HID] = inputs["We2"][l]
        wpb[:, ob + HID : ob + 2 * HID] = np.repeat(
            inputs["Wat"][l].astype(_F32), HID, axis=1
        )
    o = NLAYERS * _PL
    wp[:, o : o + HID] = inputs["W_out"].astype(_F32)
    wp[:, o + HID] = inputs["W_lin"][:, 0]
    wp[:, o + HID + 1] = inputs["b_out"]
    wp[0, o + HID + 2] = inputs["b_lin"][0]
    return wp, wcr, wpb


def _build(nc, tile_mod, bass_mod, n_layers, n_i):
    """Trace the per-core kernel into nc (a Bacc)."""
    mybir = __import__("concourse.mybir", fromlist=["mybir"])
    dt = mybir.dt.float32
    bf = mybir.dt.bfloat16
    AF = mybir.ActivationFunctionType
    ALU = mybir.AluOpType
    N2 = 2 * N

    assert n_i % 2 == 0
    npair = n_i // 2

    hT_d = nc.dram_tensor("hT0", [HID, N], dt, kind="ExternalInput")
    d2_d = nc.dram_tensor("d2p", [67, _NB * N2], bf, kind="ExternalInput")
    ma_d = nc.dram_tensor("adjp", [65, _NB * N2], bf, kind="ExternalInput")
    wp_d = nc.dram_tensor("wpack", [HID, _W_COLS], dt, kind="ExternalInput")
    wc_d = nc.dram_tensor("wcrows", [67, NLAYERS * HID], bf, kind="ExternalInput")
    wb_d = nc.dram_tensor("wpackb", [HID, _WB_COLS], bf, kind="ExternalInput")
    out_d = nc.dram_tensor("out", [1, NS], dt, kind="ExternalOutput")

    with tile_mod.TileContext(nc) as tc:
        with (
            tc.tile_pool(name="const", bufs=1) as cpool,
            tc.tile_pool(name="layer", bufs=2) as lpool,
            tc.tile_pool(name="work", bufs=4) as wpool,
            tc.tile_pool(name="psA", bufs=3, space="PSUM") as psA,
            tc.tile_pool(name="psB", bufs=3, space="PSUM") as psB,
            tc.tile_pool(name="psD", bufs=2, space="PSUM") as psD,
        ):
            # ---- load constants ----
            hT = cpool.tile([HID, N], dt, tag="hT0")
            d2p = cpool.tile([67, _NB * N2], bf, tag="d2p")
            adjp = cpool.tile([65, _NB * N2], bf, tag="adjp")
            wp = cpool.tile([HID, _W_COLS], dt, tag="wpack")
            wcr = cpool.tile([67, NLAYERS * HID], bf, tag="wcrows")
            wpb = cpool.tile([HID, _WB_COLS], bf, tag="wpackb")
            nc.sync.dma_start(hT[:], hT_d.ap())
            nc.sync.dma_start(d2p[:], d2_d.ap())
            nc.sync.dma_start(adjp[:], ma_d.ap())
            nc.sync.dma_start(wp[:], wp_d.ap())
            nc.sync.dma_start(wcr[:], wc_d.ap())
            nc.sync.dma_start(wpb[:], wb_d.ap())
            ones1_65 = cpool.tile([65, HID], bf, tag="ones1")
            nc.vector.memset(ones1_65[:], 1.0)

            hT_cur = hT
            for l in range(n_layers):
                o = l * _PL
                wa = wp[:, o : o + HID]
                wb = wp[:, o + HID : o + 2 * HID]
                Wn1a = wp[:, o + 3 * HID : o + 4 * HID]
                Wn1b = wp[:, o + 4 * HID : o + 5 * HID]
                Wn2 = wp[:, o + 5 * HID : o + 6 * HID]
                c = o + 6 * HID
                be1 = wp[:, c + 1 : c + 2]
                be2 = wp[:, c + 2 : c + 3]
                bn1 = wp[:, c + 3 : c + 4]
                bn2 = wp[:, c + 4 : c + 5]
                bat65 = wp[:, c + 5 : c + 6]
                u_last = -1
                ob = l * _PLB
                We2b = wpb[:, ob : ob + HID]

                # ---- per-layer node projections ----
                ps_ha = psA.tile([HID, N], dt, tag="pre")
                nc.tensor.matmul(ps_ha[:], wa, hT_cur[:], start=True, stop=True)
                haT = lpool.tile([HID, N], dt, tag="haT")
                nc.scalar.activation(haT[:], ps_ha[:], AF.Identity, bias=be1)
                ps_hb = psA.tile([HID, N], dt, tag="pre")
                nc.tensor.matmul(ps_hb[:], wb, hT_cur[:], start=True, stop=True)
                hbT2 = lpool.tile([HID, N2], dt, tag="hbT2")
                nc.vector.tensor_copy(hbT2[:, 0:N], ps_hb[:])
                nc.vector.tensor_copy(hbT2[:, N:N2], ps_hb[:])

                aggT = lpool.tile([HID, N], dt, tag="aggT")
                if n_i < N:
                    nc.vector.memset(aggT[:], 0.0)

                # ---- edge rows: groups of 3 pairs, stage-ordered; the
                # gating/aggregation tail of each group is emitted one group
                # late so the PE never stalls on the end of a group's chain.
                for t0 in range(0, npair, 3):
                    ts = [t for t in range(t0, min(t0 + 3, npair))]
                    bs = [32 * (t % 3) for t in ts]
                    cs = [(t // 3) * N2 for t in ts]

                    ps_pre, pre2, rpre, ps_m1, m = {}, {}, {}, {}, {}
                    for k, t in enumerate(ts):
                        b, cc = bs[k], cs[k]
                        wc3 = wcr[b : b + 3, l * HID : (l + 1) * HID]
                        ps_pre[k] = psA.tile([HID, N2], dt, tag="pre", name=f"pspre{k}")
                        nc.tensor.matmul(
                            ps_pre[k][:],
                            wc3,
                            d2p[b : b + 3, cc : cc + N2],
                            start=True,
                            stop=True,
                        )
                    for k, t in enumerate(ts):
                        pre2[k] = wpool.tile([HID, N2], dt, tag="pre2", name=f"pre2_{k}")
                        for h in range(2):
                            i = 2 * t + h
                            nc.vector.scalar_tensor_tensor(
                                out=pre2[k][:, h * N : (h + 1) * N],
                                in0=ps_pre[k][:, h * N : (h + 1) * N],
                                scalar=haT[:, i : i + 1],
                                in1=hbT2[:, 0:N],
                                op0=ALU.add,
                                op1=ALU.add,
                            )
                    for k, t in enumerate(ts):
                        rpre[k] = wpool.tile([HID, N2], bf, tag="rpre", name=f"rpre{k}")
                        nc.scalar.activation(
                            rpre[k][:], pre2[k][:], AF.Relu, bias=0.0
                        )
                    for k, t in enumerate(ts):
                        ps_m1[k] = psB.tile([HID, N2], dt, tag="m1", name=f"psm1_{k}")
                        nc.tensor.matmul(
                            ps_m1[k][:], We2b, rpre[k][:], start=True, stop=True
                        )
                    for k, t in enumerate(ts):
                        m[k] = wpool.tile([HID, N2], bf, tag="m", name=f"m{k}", bufs=8)
                        nc.scalar.activation(
                            m[k][:], ps_m1[k][:], AF.Relu, bias=be2
                        )
                    for k, t in enumerate(ts):
                        b, cc = bs[k], cs[k]
                        WatF = wpb[:, ob + HID : ob + 2 * HID]
                        ps_att = psD.tile(
                            [HID, N2], dt, tag="att", name=f"psatt{k}"
                        )
                        nc.tensor.matmul(
                            ps_att[:], WatF, m[k][:], start=True, stop=False
                        )
                        nc.tensor.matmul(
                            ps_att[:],
                            ones1_65[b : b + 1, :],
                            adjp[b : b + 1, cc : cc + N2],
                            start=False,
                            stop=True,
                        )
                        sigp = wpool.tile(
                            [HID, N2], bf, tag="sigp", name=f"sigp{k}"
                        )
                        nc.scalar.activation(
                            sigp[:], ps_att[:], AF.Sigmoid, bias=bat65
                        )
                        for h in range(2):
                            i = 2 * t + h
                            mg = wpool.tile([HID, N], dt, tag="mg", name=f"mg{k}{h}")
                            nc.vector.scalar_tensor_tensor(
                                out=mg[:],
                                in0=m[k][:, h * N : (h + 1) * N],
                                scalar=1.0 / NORM,
                                in1=sigp[:, h * N : (h + 1) * N],
                                op0=ALU.mult,
                                op1=ALU.mult,
                                accum_out=aggT[:, i : i + 1],
                            )

                # ---- node MLP:  h += relu([h, agg] @ Wn1 + bn1) @ Wn2 + bn2 ----
                ps_n1 = psA.tile([HID, N], dt, tag="pre")
                nc.tensor.matmul(ps_n1[:], Wn1a, hT_cur[:], start=True, stop=False)
                nc.tensor.matmul(ps_n1[:], Wn1b, aggT[:], start=False, stop=True)
                t1 = wpool.tile([HID, N], dt, tag="pre2")
                nc.scalar.activation(t1[:], ps_n1[:], AF.Relu, bias=bn1)
                ps_n2 = psB.tile([HID, N], dt, tag="m1")
                nc.tensor.matmul(ps_n2[:], Wn2, t1[:], start=True, stop=True)
                hsum = wpool.tile([HID, N], dt, tag="pre2")
                nc.vector.tensor_tensor(hsum[:], ps_n2[:], hT_cur[:], ALU.add)
                hT_new = lpool.tile([HID, N], dt, tag="hT")
                nc.scalar.activation(hT_new[:], hsum[:], AF.Identity, bias=bn2)
                hT_cur = hT_new

            # ---- output head ----
            o = NLAYERS * _PL
            W_out = wp[:, o : o + HID]
            W_lin = wp[:, o + HID : o + HID + 1]
            b_out = wp[:, o + HID + 1 : o + HID + 2]
            b_lin = wp[0:1, o + HID + 2 : o + HID + 3]
            ps_o = psA.tile([HID, NS], dt, tag="pre")
            nc.tensor.matmul(ps_o[:], W_out, hT_cur[:, 0:NS], start=True, stop=True)
            ho = wpool.tile([HID, NS], dt, tag="pre2")
            nc.scalar.activation(ho[:], ps_o[:], AF.Relu, bias=b_out)
            ps_y = psD.tile([1, NS], dt, tag="att")
            nc.tensor.matmul(ps_y[:], W_lin, ho[:], start=True, stop=True)
            y = wpool.tile([1, NS], dt, tag="sig3")
            nc.scalar.activation(y[:], ps_y[:], AF.Identity, bias=b_lin)
            nc.sync.dma_start(out_d.ap(), y[:])


def _make_in_maps(inputs, n_layers, n_i):
    import ml_dtypes

    h0, d2, madj, mask = _host_prep(inputs)
    wp, wcr, wpb = _pack_weights(inputs)
    wcr3 = _pack_wc_split(wcr)
    in_maps = []
    for b in range(B):
        in_maps.append(
            {
                "hT0": np.ascontiguousarray(h0[b].T),
                "d2p": _pack_d2_split(d2[b]),
                "adjp": _pack_pairs3(madj[b], ml_dtypes.bfloat16),
                "wpack": wp,
                "wcrows": wcr3,
                "wpackb": wpb,
            }
        )
    return in_maps, mask


def _install_ntff_hook():
    """Recreate the antenv.axon_hooks module the boot expected, register the
    ctypes NTFF hook from trn_agent_boot, so run_bass_kernel_spmd(trace=True)
    can capture hardware profiles under axon."""
    import types

    if "antenv.axon_hooks" not in sys.modules:
        mod = types.ModuleType("antenv.axon_hooks")
        holder = [None]
        mod.set_axon_ntff_profile_hook = lambda h: holder.__setitem__(0, h)
        mod.get_axon_ntff_profile_hook = lambda: holder[0]
        sys.modules["antenv.axon_hooks"] = mod
        import antenv

        antenv.axon_hooks = mod
    m = sys.modules["antenv.axon_hooks"]
    if m.get_axon_ntff_profile_hook() is None:
        sys.path.insert(0, "/root/.axon_site")
        from trn_agent_boot.trn_boot import _ntff_profile_via_ctypes

        m.set_axon_ntff_profile_hook(
            _ntff_profile_via_ctypes("/opt/axon/libaxon_pjrt.so")
        )


_CACHE = {}


def _get_nc(n_layers, n_i):
    key = (n_layers, n_i)
    if key not in _CACHE:
        import concourse.bass as bass
        import concourse.tile as tile
        from concourse import bacc

        nc = bacc.Bacc(
            "TRN2", target_bir_lowering=False, debug=False, num_devices=B
        )
        _build(nc, tile, bass, n_layers, n_i)
        nc.compile()
        _CACHE[key] = nc
    return _CACHE[key]


def kernel(**inputs):
    inputs = {k: np.asarray(v) for k, v in inputs.items()}
    n_layers = int(os.environ.get("GNN_LAYERS", NLAYERS))
    n_i = int(os.environ.get("GNN_NI", N))
    in_maps, mask = _make_in_maps(inputs, n_layers, n_i)
    nc = _get_nc(n_layers, n_i)

    if os.environ.get("GNN_SIM"):
        from concourse.bass_interp import CoreSim

        sim = CoreSim(nc, trace=False)
        outs = []
        for b in range(int(os.environ.get("GNN_SIM_CORES", 1))):
            for k, v in in_maps[b].items():
                sim.tensor(k)[:] = v
            sim.simulate()
            outs.append(np.array(sim.tensor("out")).reshape(NS, 1))
        while len(outs) < B:
            outs.append(np.zeros((NS, 1), _F32))
        out = np.stack(outs)
    else:
        from concourse.bass_utils import run_bass_kernel_spmd

        if os.environ.get("GNN_TRACE"):
            _install_ntff_hook()
            tmpdir = os.environ.get("GNN_TRACE_DIR") or None
            try:
                res = run_bass_kernel_spmd(
                    nc, in_maps, core_ids=list(range(B)), trace=True, tmpdir=tmpdir
                )
                kernel.last_exec_time_ns = res.exec_time_ns
            except Exception as e:
                print(f"[gnn] traced run failed ({e!r}); retrying untraced")
                res = run_bass_kernel_spmd(nc, in_maps, core_ids=list(range(B)))
        else:
            res = run_bass_kernel_spmd(nc, in_maps, core_ids=list(range(B)))
        kernel.last_results = res
        out = np.stack([r["out"].reshape(NS, 1) for r in res.results])

    return (out * inputs["node_mask"][:, :, None]).astype(_F32)


# revision 24
# speedup vs baseline: 1.2952x; 1.0306x over previous
"""Trainium2 Bass kernel for AnchorGNNPocket (GNN message passing).

Data-parallel over batch B=8: one complex per NeuronCore. Each core runs the
full 4-layer GCL stack on its sample.

v2: edge rows processed in PAIRS ([128, 512] tiles = one PSUM bank), emitted
stage-by-stage over groups of 3 pairs (6 rows) so each engine sees batches of
independent work (software pipelining); edge-MLP matmuls in bf16 (fp32 matmul
is 2-pass LOW_HIGH on trn2); sigmoid batched over 3 pairs by landing the three
attention rows on PSUM partitions {0,32,64} via column-embedded Wat
stationaries.

Per edge-row i the math is
  pre      = wc (x) d2[i,:] + hb + ha[:,i]     (K=1 fp32 matmul + DVE + ScalarE)
  m        = relu(We2^T relu(pre) + be2)       (bf16 matmul + ScalarE)
  att      = Wat^T m + madj[i,:] + bat         (bf16 matmuls; madj = -1e9 on
                                                non-edges => sigmoid == adj)
  w        = sigmoid(att)                      (ScalarE, batched over 6 rows)
  agg[:,i] = sum_j (m/100) * bcast(w)          (ones-matmul + DVE
                                                scalar_tensor_tensor fused)
"""

import os
import sys

import numpy as np

if not any(os.path.isdir(os.path.join(p, "concourse")) for p in sys.path if p):
    sys.path.insert(0, "/opt/trn_rl_repo")

# ---- problem constants (hardcoded per contest rules) ----
B, NS, NP = 8, 32, 224
N = NS + NP                      # 256 nodes
LIG_NF, POK_NF, JNF, HID, OUT_NF, NLAYERS = 10, 25, 32, 128, 128, 4
CUT2 = 4.5 ** 2
NORM = 100.0

_F32 = np.float32
NPAIR = N // 2                   # 128 row-pairs
_NB = (NPAIR + 2) // 3           # pair slots per base partition (43)


def _np_silu(x):
    return x / (1.0 + np.exp(-x))


def _host_prep(inputs):
    """Host-side preprocessing: embedding h0, pairwise d2, adjacency logits."""
    x = np.concatenate([inputs["mol_x"], inputs["pocket_x"]], axis=1).astype(_F32)
    mask = np.concatenate([inputs["node_mask"], inputs["pocket_mask"]], axis=1).astype(
        _F32
    )
    hm = _np_silu(inputs["mol_h"].astype(_F32) @ inputs["W_mol"] + inputs["b_mol"])
    hp = _np_silu(
        inputs["pocket_h"].astype(_F32) @ inputs["W_pok"] + inputs["b_pok"]
    )
    h0 = (
        np.concatenate([hm, hp], axis=1) @ inputs["W_emb"] + inputs["b_emb"]
    ).astype(_F32)  # [B, N, H]

    diff = x[:, :, None, :] - x[:, None, :, :]
    d2 = np.sum(diff * diff, axis=-1, dtype=_F32)  # [B, N, N]
    idx = np.arange(N)
    lig_pair = (idx[:, None] < NS) & (idx[None, :] < NS)
    adj = np.where(lig_pair, 1.0, (d2 <= CUT2).astype(_F32))
    adj = adj * mask[:, :, None] * mask[:, None, :]
    madj = np.where(adj > 0, 0.0, -1.0e9).astype(_F32)
    return h0, d2, madj, mask


def _pack_pairs3(mat, dtype):
    """[256, 256] -> [65, 43*512]: row pair (2t, 2t+1) at partition 32*(t%3),
    cols (t//3)*512. Matmul rhs APs must start at partition 0/32/64."""
    out = np.zeros((65, _NB * 2 * N), dtype=dtype)
    for t in range(NPAIR):
        c = (t // 3) * 2 * N
        out[32 * (t % 3), c : c + N] = mat[2 * t]
        out[32 * (t % 3), c + N : c + 2 * N] = mat[2 * t + 1]
    return out


def _pack_d2_split(d2):
    """bf16 hi/lo split of d2 rows: partition b holds d2_hi, b+1 d2_lo,
    b+2 d2_hi again (pairs with lhsT rows [wc_hi, wc_hi, wc_lo])."""
    import ml_dtypes

    bf = ml_dtypes.bfloat16
    out = np.zeros((67, _NB * 2 * N), dtype=bf)
    for t in range(NPAIR):
        b, c = 32 * (t % 3), (t // 3) * 2 * N
        row = np.concatenate([d2[2 * t], d2[2 * t + 1]]).astype(_F32)
        hi = row.astype(bf)
        lo = (row - hi.astype(_F32)).astype(bf)
        out[b, c : c + 2 * N] = hi
        out[b + 1, c : c + 2 * N] = lo
        out[b + 2, c : c + 2 * N] = hi
    return out


def _pack_wc_split(wcr):
    """[1, L*H] f32 -> [67, L*H] bf16 with rows [wc_hi, wc_hi, wc_lo] at each
    base partition in {0, 32, 64}."""
    import ml_dtypes

    bf = ml_dtypes.bfloat16
    hi = wcr.astype(bf)
    lo = (wcr.astype(_F32) - hi.astype(_F32)).astype(bf)
    out = np.zeros((67, wcr.shape[-1]), dtype=bf)
    for b in (0, 32, 64):
        out[b] = hi
        out[b + 1] = hi
        out[b + 2] = lo
    return out


def _rep3(row):
    out = np.zeros((65, row.shape[-1]), dtype=row.dtype)
    out[0] = out[32] = out[64] = row
    return out


# weight-pack column offsets (per layer stride)
_PL = 6 * HID + 8  # wa, wb, We2, Wn1a, Wn1b, Wn2 (128 each) + small cols
_W_COLS = NLAYERS * _PL + HID + 8  # + W_out + wlin/bout/blin
_PLB = 2 * HID  # bf16 pack per layer: We2 | WatFull (Wat in all 128 cols)
_WB_COLS = NLAYERS * _PLB


def _pack_weights(inputs):
    import ml_dtypes

    wp = np.zeros((HID, _W_COLS), dtype=_F32)
    wcr = np.zeros((1, NLAYERS * HID), dtype=_F32)
    wpb = np.zeros((HID, _WB_COLS), dtype=ml_dtypes.bfloat16)
    We1 = inputs["We1"].astype(_F32)
    for l in range(NLAYERS):
        o = l * _PL
        wp[:, o : o + HID] = We1[l, :HID, :]              # wa
        wp[:, o + HID : o + 2 * HID] = We1[l, HID : 2 * HID, :]  # wb
        wp[:, o + 2 * HID : o + 3 * HID] = inputs["We2"][l]
        wp[:, o + 3 * HID : o + 4 * HID] = inputs["Wn1"][l][:HID, :]
        wp[:, o + 4 * HID : o + 5 * HID] = inputs["Wn1"][l][HID:, :]
        wp[:, o + 5 * HID : o + 6 * HID] = inputs["Wn2"][l]
        c = o + 6 * HID
        wp[:, c + 1] = inputs["be1"][l]
        wp[:, c + 2] = inputs["be2"][l]
        wp[:, c + 3] = inputs["bn1"][l]
        wp[:, c + 4] = inputs["bn2"][l]
        wp[:, c + 5] = inputs["bat"][l][0]                # bat replicated
        wcr[0, l * HID : (l + 1) * HID] = We1[l, 2 * HID, :]
        ob = l * _PLB
        wpb[:, ob : ob + HID] = inputs["We2"][l]
        wpb[:, ob + HID : ob + 2 * HID] = np.repeat(
            inputs["Wat"][l].astype(_F32), HID, axis=1
        )
    o = NLAYERS * _PL
    wp[:, o : o + HID] = inputs["W_out"].astype(_F32)
    wp[:, o + HID] = inputs["W_lin"][:, 0]
    wp[:, o + HID + 1] = inputs["b_out"]
    wp[0, o + HID + 2] = inputs["b_lin"][0]
    return wp, wcr, wpb


def _build(nc, tile_mod, bass_mod, n_layers, n_i):
    """Trace the per-core kernel into nc (a Bacc)."""
    mybir = __import__("concourse.mybir", fromlist=["mybir"])
    dt = mybir.dt.float32
    bf = mybir.dt.bfloat16
    AF = mybir.ActivationFunctionType
    ALU = mybir.AluOpType
    N2 = 2 * N

    assert n_i % 2 == 0
    npair = n_i // 2

    hT_d = nc.dram_tensor("hT0", [HID, N], dt, kind="ExternalInput")
    d2_d = nc.dram_tensor("d2p", [67, _NB * N2], bf, kind="ExternalInput")
    ma_d = nc.dram_tensor("adjp", [65, _NB * N2], bf, kind="ExternalInput")
    wp_d = nc.dram_tensor("wpack", [HID, _W_COLS], dt, kind="ExternalInput")
    wc_d = nc.dram_tensor("wcrows", [67, NLAYERS * HID], bf, kind="ExternalInput")
    wb_d = nc.dram_tensor("wpackb", [HID, _WB_COLS], bf, kind="ExternalInput")
    out_d = nc.dram_tensor("out", [1, NS], dt, kind="ExternalOutput")

    with tile_mod.TileContext(nc) as tc:
        with (
            tc.tile_pool(name="const", bufs=1) as cpool,
            tc.tile_pool(name="layer", bufs=2) as lpool,
            tc.tile_pool(name="work", bufs=4) as wpool,
            tc.tile_pool(name="psA", bufs=3, space="PSUM") as psA,
            tc.tile_pool(name="psB", bufs=3, space="PSUM") as psB,
            tc.tile_pool(name="psD", bufs=2, space="PSUM") as psD,
        ):
            # ---- load constants ----
            hT = cpool.tile([HID, N], dt, tag="hT0")
            d2p = cpool.tile([67, _NB * N2], bf, tag="d2p")
            adjp = cpool.tile([65, _NB * N2], bf, tag="adjp")
            wp = cpool.tile([HID, _W_COLS], dt, tag="wpack")
            wcr = cpool.tile([67, NLAYERS * HID], bf, tag="wcrows")
            wpb = cpool.tile([HID, _WB_COLS], bf, tag="wpackb")
            nc.sync.dma_start(hT[:], hT_d.ap())
            nc.sync.dma_start(d2p[:], d2_d.ap())
            nc.sync.dma_start(adjp[:], ma_d.ap())
            nc.sync.dma_start(wp[:], wp_d.ap())
            nc.sync.dma_start(wcr[:], wc_d.ap())
            nc.sync.dma_start(wpb[:], wb_d.ap())
            ones1_65 = cpool.tile([65, HID], bf, tag="ones1")
            nc.vector.memset(ones1_65[:], 1.0)

            hT_cur = hT
            for l in range(n_layers):
                o = l * _PL
                wa = wp[:, o : o + HID]
                wb = wp[:, o + HID : o + 2 * HID]
                Wn1a = wp[:, o + 3 * HID : o + 4 * HID]
                Wn1b = wp[:, o + 4 * HID : o + 5 * HID]
                Wn2 = wp[:, o + 5 * HID : o + 6 * HID]
                c = o + 6 * HID
                be1 = wp[:, c + 1 : c + 2]
                be2 = wp[:, c + 2 : c + 3]
                bn1 = wp[:, c + 3 : c + 4]
                bn2 = wp[:, c + 4 : c + 5]
                bat65 = wp[:, c + 5 : c + 6]
                u_last = -1
                ob = l * _PLB
                We2b = wpb[:, ob : ob + HID]

                # ---- per-layer node projections ----
                ps_ha = psA.tile([HID, N], dt, tag="pre")
                nc.tensor.matmul(ps_ha[:], wa, hT_cur[:], start=True, stop=True)
                haT = lpool.tile([HID, N], dt, tag="haT")
                nc.scalar.activation(haT[:], ps_ha[:], AF.Identity, bias=be1)
                ps_hb = psA.tile([HID, N], dt, tag="pre")
                nc.tensor.matmul(ps_hb[:], wb, hT_cur[:], start=True, stop=True)
                hbT2 = lpool.tile([HID, N2], dt, tag="hbT2")
                nc.vector.tensor_copy(hbT2[:, 0:N], ps_hb[:])
                nc.vector.tensor_copy(hbT2[:, N:N2], ps_hb[:])

                aggT = lpool.tile([HID, N], dt, tag="aggT")
                if n_i < N:
                    nc.vector.memset(aggT[:], 0.0)

                # ---- edge rows: groups of 3 pairs, stage-ordered; the
                # gating/aggregation tail of each group is emitted one group
                # late so the PE never stalls on the end of a group's chain.
                for t0 in range(0, npair, 3):
                    ts = [t for t in range(t0, min(t0 + 3, npair))]
                    bs = [32 * (t % 3) for t in ts]
                    cs = [(t // 3) * N2 for t in ts]

                    ps_pre, pre2, rpre, ps_m1, m = {}, {}, {}, {}, {}
                    for k, t in enumerate(ts):
                        b, cc = bs[k], cs[k]
                        wc3 = wcr[b : b + 3, l * HID : (l + 1) * HID]
                        ps_pre[k] = psA.tile([HID, N2], dt, tag="pre", name=f"pspre{k}")
                        nc.tensor.matmul(
                            ps_pre[k][:],
                            wc3,
                            d2p[b : b + 3, cc : cc + N2],
                            start=True,
                            stop=True,
                        )
                    for k, t in enumerate(ts):
                        pre2[k] = wpool.tile([HID, N2], dt, tag="pre2", name=f"pre2_{k}")
                        for h in range(2):
                            i = 2 * t + h
                            nc.vector.scalar_tensor_tensor(
                                out=pre2[k][:, h * N : (h + 1) * N],
                                in0=ps_pre[k][:, h * N : (h + 1) * N],
                                scalar=haT[:, i : i + 1],
                                in1=hbT2[:, 0:N],
                                op0=ALU.add,
                                op1=ALU.add,
                            )
                    for k, t in enumerate(ts):
                        rpre[k] = wpool.tile([HID, N2], bf, tag="rpre", name=f"rpre{k}")
                        if (t0 // 3 + k) % 2 == 0:
                            nc.scalar.activation(
                                rpre[k][:], pre2[k][:], AF.Relu, bias=0.0
                            )
                        else:
                            nc.vector.tensor_scalar_max(
                                rpre[k][:], pre2[k][:], 0.0
                            )
                    for k, t in enumerate(ts):
                        ps_m1[k] = psB.tile([HID, N2], dt, tag="m1", name=f"psm1_{k}")
                        nc.tensor.matmul(
                            ps_m1[k][:], We2b, rpre[k][:], start=True, stop=True
                        )
                    for k, t in enumerate(ts):
                        m[k] = wpool.tile([HID, N2], bf, tag="m", name=f"m{k}", bufs=8)
                        nc.scalar.activation(
                            m[k][:], ps_m1[k][:], AF.Relu, bias=be2
                        )
                    for k, t in enumerate(ts):
                        b, cc = bs[k], cs[k]
                        WatF = wpb[:, ob + HID : ob + 2 * HID]
                        ps_att = psD.tile(
                            [HID, N2], dt, tag="att", name=f"psatt{k}"
                        )
                        nc.tensor.matmul(
                            ps_att[:], WatF, m[k][:], start=True, stop=False
                        )
                        nc.tensor.matmul(
                            ps_att[:],
                            ones1_65[b : b + 1, :],
                            adjp[b : b + 1, cc : cc + N2],
                            start=False,
                            stop=True,
                        )
                        sigp = wpool.tile(
                            [HID, N2], bf, tag="sigp", name=f"sigp{k}"
                        )
                        nc.scalar.activation(
                            sigp[:], ps_att[:], AF.Sigmoid, bias=bat65
                        )
                        for h in range(2):
                            i = 2 * t + h
                            mg = wpool.tile([HID, N], dt, tag="mg", name=f"mg{k}{h}")
                            nc.vector.scalar_tensor_tensor(
                                out=mg[:],
                                in0=m[k][:, h * N : (h + 1) * N],
                                scalar=1.0 / NORM,
                                in1=sigp[:, h * N : (h + 1) * N],
                                op0=ALU.mult,
                                op1=ALU.mult,
                                accum_out=aggT[:, i : i + 1],
                            )

                # ---- node MLP:  h += relu([h, agg] @ Wn1 + bn1) @ Wn2 + bn2 ----
                ps_n1 = psA.tile([HID, N], dt, tag="pre")
                nc.tensor.matmul(ps_n1[:], Wn1a, hT_cur[:], start=True, stop=False)
                nc.tensor.matmul(ps_n1[:], Wn1b, aggT[:], start=False, stop=True)
                t1 = wpool.tile([HID, N], dt, tag="pre2")
                nc.scalar.activation(t1[:], ps_n1[:], AF.Relu, bias=bn1)
                ps_n2 = psB.tile([HID, N], dt, tag="m1")
                nc.tensor.matmul(ps_n2[:], Wn2, t1[:], start=True, stop=True)
                hsum = wpool.tile([HID, N], dt, tag="pre2")
                nc.vector.tensor_tensor(hsum[:], ps_n2[:], hT_cur[:], ALU.add)
                hT_new = lpool.tile([HID, N], dt, tag="hT")
                nc.scalar.activation(hT_new[:], hsum[:], AF.Identity, bias=bn2)
                hT_cur = hT_new

            # ---- output head ----
            o = NLAYERS * _PL
            W_out = wp[:, o : o + HID]
            W_lin = wp[:, o + HID : o + HID + 1]
            b_out = wp[:, o + HID + 1 : o + HID + 2]
            b_lin = wp[0:1, o + HID + 2 : o + HID + 3]
            ps_o = psA.tile([HID, NS], dt, tag="pre")
            nc.tensor.matmul(ps_o[:], W_out, hT_cur[:, 0:NS], start=True, stop=True)
            ho = wpool.tile([HID, NS], dt, tag="pre2")
            nc.scalar.activation(ho[:], ps_o[:], AF.Relu, bias=b_out)
            ps_y = psD.tile([1, NS], dt, tag="att")
            nc.tensor.matmul(ps_y[:], W_lin, ho[:], start=True, stop=True)
            y = wpool.tile([1, NS], dt, tag="sig3")
            nc.scalar.activation(y[:], ps_y[:], AF.Identity, bias=b_lin)
            nc.sync.dma_start(out_d.ap(), y[:])


def _make_in_maps(inputs, n_layers, n_i):
    import ml_dtypes

    h0, d2, madj, mask = _host_prep(inputs)
    wp, wcr, wpb = _pack_weights(inputs)
    wcr3 = _pack_wc_split(wcr)
    in_maps = []
    for b in range(B):
        in_maps.append(
            {
                "hT0": np.ascontiguousarray(h0[b].T),
                "d2p": _pack_d2_split(d2[b]),
                "adjp": _pack_pairs3(madj[b], ml_dtypes.bfloat16),
                "wpack": wp,
                "wcrows": wcr3,
                "wpackb": wpb,
            }
        )
    return in_maps, mask


def _install_ntff_hook():
    """Recreate the antenv.axon_hooks module the boot expected, register the
    ctypes NTFF hook from trn_agent_boot, so run_bass_kernel_spmd(trace=True)
    can capture hardware profiles under axon."""
    import types

    if "antenv.axon_hooks" not in sys.modules:
        mod = types.ModuleType("antenv.axon_hooks")
        holder = [None]
        mod.set_axon_ntff_profile_hook = lambda h: holder.__setitem__(0, h)
        mod.get_axon_ntff_profile_hook = lambda: holder[0]
        sys.modules["antenv.axon_hooks"] = mod
        import antenv

        antenv.axon_hooks = mod
    m = sys.modules["antenv.axon_hooks"]
    if m.get_axon_ntff_profile_hook() is None:
        sys.path.insert(0, "/root/.axon_site")
        from trn_agent_boot.trn_boot import _ntff_profile_via_ctypes

        m.set_axon_ntff_profile_hook(
            _ntff_profile_via_ctypes("/opt/axon/libaxon_pjrt.so")
        )


_CACHE = {}


def _get_nc(n_layers, n_i):
    key = (n_layers, n_i)
    if key not in _CACHE:
        import concourse.bass as bass
        import concourse.tile as tile
        from concourse import bacc

        nc = bacc.Bacc(
            "TRN2", target_bir_lowering=False, debug=False, num_devices=B
        )
        _build(nc, tile, bass, n_layers, n_i)
        nc.compile()
        _CACHE[key] = nc
    return _CACHE[key]


def kernel(**inputs):
    inputs = {k: np.asarray(v) for k, v in inputs.items()}
    n_layers = int(os.environ.get("GNN_LAYERS", NLAYERS))
    n_i = int(os.environ.get("GNN_NI", N))
    in_maps, mask = _make_in_maps(inputs, n_layers, n_i)
    nc = _get_nc(n_layers, n_i)

    if os.environ.get("GNN_SIM"):
        from concourse.bass_interp import CoreSim

        sim = CoreSim(nc, trace=False)
        outs = []
        for b in range(int(os.environ.get("GNN_SIM_CORES", 1))):
            for k, v in in_maps[b].items():
                sim.tensor(k)[:] = v
            sim.simulate()
            outs.append(np.array(sim.tensor("out")).reshape(NS, 1))
        while len(outs) < B:
            outs.append(np.zeros((NS, 1), _F32))
        out = np.stack(outs)
    else:
        from concourse.bass_utils import run_bass_kernel_spmd

        if os.environ.get("GNN_TRACE"):
            _install_ntff_hook()
            tmpdir = os.environ.get("GNN_TRACE_DIR") or None
            try:
                res = run_bass_kernel_spmd(
                    nc, in_maps, core_ids=list(range(B)), trace=True, tmpdir=tmpdir
                )
                kernel.last_exec_time_ns = res.exec_time_ns
            except Exception as e:
                print(f"[gnn] traced run failed ({e!r}); retrying untraced")
                res = run_bass_kernel_spmd(nc, in_maps, core_ids=list(range(B)))
        else:
            res = run_bass_kernel_spmd(nc, in_maps, core_ids=list(range(B)))
        kernel.last_results = res
        out = np.stack([r["out"].reshape(NS, 1) for r in res.results])

    return (out * inputs["node_mask"][:, :, None]).astype(_F32)
